# revision 1
# baseline (speedup 1.0000x reference)
"""CenterPooling kernel for Trainium2 (8 NeuronCores, SPMD over batch).

Math note: for any tensor t, cummax(t, reverse=True) followed by cummax(t)
along the same axis equals broadcast(max(t, axis)) — the suffix-max is
non-increasing, so its prefix-max is the global max everywhere.  Hence:

    out[n,c,h,w] = A[n,c,h] + B[n,c,w]
    A = max_w relu(bn(conv3x3(x, w_up)))     (up branch)
    B = max_h relu(bn(conv3x3(x, w_down)))   (down branch)

BN folding: bn(y) = y*scale + shift with scale = g/sqrt(v+eps) per out
channel; scale folds into the conv weights on the host.  shift + relu are
monotone per channel, so they commute past the max and apply to the reduced
[C,H]/[C,W] tensors only.

fp8 path: matmuls run as float8e4 (e4m3) in MatmulPerfMode.DoubleRow —
2 k-tiles of 128 channels contract per matmul at 0.5 cycles/row, 4x the
bf16 rate.  Plain e4m3 quantization of both operands lands at rel err
~3.4e-2 (gate is 2e-2), so each operand is split hi+lo residual pairs:

    y = xh*wh + xh*wl + xl*wh        (xl*wl dropped; ~1e-3 rel err)

3 DoubleRow matmuls per tap instead of 2 bf16 matmuls: 13.5 vs 18
cycles/output.  x is pre-scaled by 16 and w per-cout to a power of two
(into e4m3 normal range); the activation un-scales via its scale AP —
positive scaling commutes with max.

Sharding: data-parallel over batch, 4 images per core, weights replicated.
"""

import sys

import numpy as np

for _p in ("/opt/trn_rl_repo", "/opt/pypackages"):
    if _p not in sys.path:
        sys.path.append(_p)

import concourse.bacc as bacc
import concourse.bass as bass
import concourse.mybir as mybir
import concourse.tile as tile
from concourse.bass_utils import run_bass_kernel_spmd

N_CORES = 8
B, C, H, W = 32, 256, 128, 128
BPC = B // N_CORES
EPS = 1e-5
XS = 16.0     # x pre-scale into e4m3 normal range
WMAX = 200.0  # per-cout weight pre-scale target (e4m3 max finite = 240)

F32 = mybir.dt.float32
FP8 = mybir.dt.float8e4
DR = mybir.MatmulPerfMode.DoubleRow


def build_program(bpc: int = BPC, h: int = H, grp: int = 4) -> bass.Bass:
    """Build the per-core Bass program.

    Inputs (per core):
      x     [bpc, 128, 2, 2, HP*WP] fp8  hi/lo split, k = ci//128 tiles
      wq    [128, 72, 2, 128] fp8        packed conv weights (see pack_weights)
      bias  [128, 4] f32                 bn shifts per (branch, cout-tile)
      scale [128, 4] f32                 1/(XS*sw) per (branch, cout-tile)
    Output:
      out   [bpc, C, h, W] f32
    """
    WP = W + 2            # padded width  (zero cols at 0 and W+1)
    HP = h + 2            # padded height (zero rows at 0 and h+1)
    RELU = mybir.ActivationFunctionType.Relu
    AX = mybir.AxisListType.X

    nc = bacc.Bacc("TRN2", debug=False, enable_asserts=False)
    x_d = nc.dram_tensor("x", [bpc, 128, 2, 2, HP, WP], FP8,
                         kind="ExternalInput")
    wq_d = nc.dram_tensor("wq", [128, 72, 2, 128], FP8, kind="ExternalInput")
    bias_d = nc.dram_tensor("bias", [128, 4], F32, kind="ExternalInput")
    scale_d = nc.dram_tensor("scale", [128, 4], F32, kind="ExternalInput")
    out_d = nc.dram_tensor("out", [bpc, C, h, W], F32, kind="ExternalOutput")
    xa, wa, oa = x_d.ap(), wq_d.ap(), out_d.ap()

    with tile.TileContext(nc) as tc:
        with (
            tc.tile_pool(name="wts", bufs=1) as wpool,
            tc.tile_pool(name="xpad", bufs=2) as xpool,
            tc.tile_pool(name="psum", bufs=8, space="PSUM") as ppool,
            tc.tile_pool(name="red", bufs=3) as rpool,
            tc.tile_pool(name="outp", bufs=3) as opool,
        ):
            # warmup: dummy matmuls on a memset region keep the PE busy
            # through the startup DMA wait so the p-state ramp (3us to
            # full clock) completes before the first real matmul; sized to
            # just undershoot the ~4.4us band-A DMA wait
            warm = wpool.tile([128, 2, 512], FP8, name="warm")
            nc.vector.memset(warm[:], 0.0)
            wpt = ppool.tile([128, 512], F32, tag="ps", name="warm_ps")
            for r in range(36):
                nc.tensor.matmul(wpt[:], warm[:, :, 0:128], warm[:],
                                 start=True, stop=True, perf_mode=DR)

            wq_sb = wpool.tile([128, 72, 2, 128], FP8, name="wq_sb")
            # first unit's weights gate the first matmul: DMA them alone
            # ahead of the big x transfer on the serial DMA device
            nc.sync.dma_start(wq_sb[:, 0:18], wa[:, 0:18, :, :])
            bias_sb = wpool.tile([128, 4], F32, name="bias_sb")
            scale_sb = wpool.tile([128, 4], F32, name="scale_sb")

            # down-branch units run first so B[c,w] is ready early; the up
            # units then emit fin rows + output blocks per chunk group, so
            # the assembly pipeline drains with (not after) the matmuls
            UNITS = ((1, 0), (1, 1), (0, 0), (0, 1))
            RPC = 4                       # rows per chunk (4*128 = one bank)
            GR = grp * RPC                # rows per chunk group
            HB = 8                        # output rows per DMA block
            y0s = list(range(0, h, RPC))

            # x arrives in two row bands per split (A: padded rows 0..29,
            # B: rows 28..129, 2-row overlap).  Chunks never straddle the
            # boundary (y0 <= 24 reads rows <= 29; y0 >= 28 reads >= 28),
            # and image 0's first matmul waits only on the small band-A
            # transfer instead of the whole plane on the serial DMA device.
            ROWS_A, ROW_B0 = 30, 28
            for n in range(bpc):
                # ---- load image n: 4 band DMAs (hiA, loA, hiB, loB) ----
                xb = {}
                for s in range(2):
                    xb[(s, 0)] = xpool.tile([128, 2, ROWS_A, WP], FP8,
                                            tag=f"xA{s}", name=f"xA{s}_{n}")
                    xb[(s, 1)] = xpool.tile([128, 2, HP - ROW_B0, WP], FP8,
                                            tag=f"xB{s}", name=f"xB{s}_{n}")
                # issue order matters on the serial DMA device: image 0's
                # first matmuls wait only on hiA, the xl terms on loA.  Each
                # band's DMA wait lands on the first matmul that reads it
                # (move_matmul_waits_to_ldweights parks it on the ldweights).
                for s, b in ((0, 0), (1, 0), (0, 1), (1, 1)):
                    rows = slice(0, ROWS_A) if b == 0 else slice(ROW_B0, HP)
                    nc.sync.dma_start(xb[(s, b)][:], xa[n, :, s, :, rows, :])
                if n == 0:
                    # the bulk transfers queue behind image 0's x on the
                    # serial DMA device; nothing needs them until ~90us in
                    nc.sync.dma_start(wq_sb[:, 18:72], wa[:, 18:72, :, :])
                    nc.sync.dma_start(bias_sb[:], bias_d.ap()[:, :])
                    nc.sync.dma_start(scale_sb[:], scale_d.ap()[:, :])

                fins = {}
                for u, (br, co) in enumerate(UNITS):
                    if br == 0:
                        acc = rpool.tile([128, h], F32, tag="Araw",
                                         name=f"Araw_{n}_{co}")
                        fin = rpool.tile([128, h], F32, tag="Af", bufs=4,
                                         name=f"fin_{n}_{br}_{co}")
                    else:
                        acc = rpool.tile([128, W], F32, tag="Braw",
                                         name=f"Braw_{n}_{co}")
                        nc.vector.memset(acc[:], -3.0e38)
                        fin = rpool.tile([128, W], F32, tag="Bf", bufs=4,
                                         name=f"fin_{n}_{br}_{co}")
                    fins[(br, co)] = fin
                    bcol = br * 2 + co
                    # chunk groups share one LDWEIGHTS per weight (the
                    # duplicate loads are deleted by _dedup_ldweights);
                    # grp=4 of the 8 PSUM banks keeps two groups in
                    # flight so the end-of-group DVE reduce burst hides
                    # under the next group's matmuls
                    # 6 of the 18 correction matmuls are dropped: the
                    # measured rel err rises 0.0009 -> 0.0161 (gate is
                    # 0.02) and PE work falls to 21/27.  x-split major so
                    # the xl matmuls come last (image 0 waits on loA less).
                    terms = [(xs, ws, d)
                             for xs, wss in ((0, (0, 1)), (1, (0,)))
                             for ws in wss
                             for d in range(9)
                             if not ((xs, ws) == (0, 1) and d in (0, 6, 8))
                             and not ((xs, ws) == (1, 0) and d in (0, 2, 4))]

                    def rhs_ap(xs, y0, dy, dx):
                        if y0 <= 24:
                            t, r0 = xb[(xs, 0)], y0 + dy
                        else:
                            t, r0 = xb[(xs, 1)], y0 + dy - ROW_B0
                        return t[:, :, r0:r0 + RPC, dx:dx + W]

                    last_g0 = len(y0s) - grp
                    for g0 in range(0, len(y0s), grp):
                        cgrp = y0s[g0:g0 + grp]
                        pts = [ppool.tile([128, RPC * W], F32, tag="ps",
                                          name=f"ps_{n}_{br}_{co}_{y0}")
                               for y0 in cgrp]
                        if n == bpc - 1 and u == 3 and g0 == last_g0:
                            # final group: chunk-major so each chunk's
                            # reduce pipelines with the next chunk's
                            # matmuls instead of bursting after the last
                            order = [(xs, ws, d, k) for k in range(grp)
                                     for (xs, ws, d) in terms]
                        else:
                            order = [(xs, ws, d, k) for (xs, ws, d) in terms
                                     for k in range(grp)]
                        for xs, ws, d, k in order:
                            dy, dx = divmod(d, 3)
                            wap = wq_sb[:, (u * 9 + d) * 2 + ws, :, :]
                            nc.tensor.matmul(
                                pts[k][:], wap, rhs_ap(xs, y0s[g0] + 4 * k,
                                                       dy, dx),
                                start=(xs == 0 and ws == 0 and d == 0),
                                stop=(xs == 1 and d == 8),
                                perf_mode=DR)
                        for k, y0 in enumerate(cgrp):
                            pv = pts[k].rearrange("p (r x) -> p r x", x=W)
                            if br == 0:
                                # max over w within each row
                                nc.vector.reduce_max(acc[:, y0:y0 + RPC],
                                                     pv, axis=AX)
                            else:
                                # max over rows per column, then running
                                # max across row-chunks
                                cm = rpool.tile([128, W], F32, tag="cm",
                                                bufs=4,
                                                name=f"cm_{n}_{co}_{y0}")
                                nc.vector.reduce_max(
                                    cm[:], pv.transpose([0, 2, 1]), axis=AX)
                                nc.vector.tensor_max(acc[:], acc[:], cm[:])
                        if br == 0:
                            # fin rows for this group, then their output
                            # blocks: out[c, y, x] = A[c, y] + B[c, x],
                            # split across the Act and DVE engines.  The
                            # very last group runs at half-group (8-row)
                            # granularity so the final block drains right
                            # behind the final chunk-reduce.
                            gy = g0 * RPC
                            b_f = fins[(1, co)]
                            tail = (n == bpc - 1 and u == 3
                                    and g0 == last_g0)
                            step = HB if tail else GR
                            for a0 in range(gy, gy + GR, step):
                                nc.scalar.activation(
                                    fin[:, a0:a0 + step],
                                    acc[:, a0:a0 + step], RELU,
                                    bias=bias_sb[:, bcol:bcol + 1],
                                    scale=scale_sb[:, bcol:bcol + 1])
                                for hb in range(a0, a0 + step, HB):
                                    ot = opool.tile([128, HB, W], F32,
                                                    tag="ot",
                                                    name=f"ot_{n}_{co}_{hb}")
                                    for j in range(HB):
                                        if (j % 2 == 0) != tail:
                                            nc.scalar.add(
                                                ot[:, j, :], b_f[:],
                                                fin[:, hb + j:hb + j + 1])
                                        else:
                                            nc.vector.tensor_scalar_add(
                                                ot[:, j, :], b_f[:],
                                                fin[:, hb + j:hb + j + 1])
                                    csl = slice(co * 128, (co + 1) * 128)
                                    if tail:
                                        for q in (0, HB // 2):
                                            nc.sync.dma_start(
                                                oa[n, csl,
                                                   hb + q:hb + q + HB // 2,
                                                   :],
                                                ot[:, q:q + HB // 2, :])
                                    else:
                                        nc.sync.dma_start(
                                            oa[n, csl, hb:hb + HB, :], ot[:])
                    if br == 1:
                        nc.scalar.activation(fin[:], acc[:], RELU,
                                             bias=bias_sb[:, bcol:bcol + 1],
                                             scale=scale_sb[:, bcol:bcol + 1])
    _dedup_ldweights(nc)
    nc.compile()
    return nc


def _dedup_ldweights(nc) -> int:
    """Delete consecutive InstLdweights that reload identical weights.

    Tile lowering emits one LDWEIGHTS per matmul even when the stationary
    operand is unchanged.  The PE keeps the stationary operand between
    matmuls, and the non-self-loading InstMatmult still carries the weights
    AP in ins[1], so dropping an exact-duplicate reload is semantics
    preserving.  Only waits/updates-free duplicates are removed, and any
    other PE instruction resets the tracked state (conservative).
    """
    removed = 0
    for bb in nc.m.functions[0].blocks:
        last_key = None
        keep = []
        for inst in bb.instructions:
            tn = type(inst).__name__
            if getattr(inst, "engine", None) == mybir.EngineType.PE:
                if tn == "InstLdweights":
                    si = inst.sync_info
                    clean = si is None or (not si.on_wait and not si.on_update)
                    key = repr(inst.ins[0])
                    if clean and last_key == key:
                        removed += 1
                        continue  # drop exact-duplicate reload
                    last_key = key
                elif tn != "InstMatmult":
                    # unknown PE instruction: assume weights state clobbered
                    last_key = None
            keep.append(inst)
        bb.instructions[:] = keep
    return removed


def _q8(a: np.ndarray) -> np.ndarray:
    import ml_dtypes
    return a.astype(ml_dtypes.float8_e4m3)


def pack_weights(w: np.ndarray, gamma: np.ndarray, var: np.ndarray):
    """Fold BN scale into OIHW conv weights, emit fp8 hi/lo DoubleRow layout.

    Returns (wq [128, 2(co), 9(tap), 2(ws), 2(k), 128(m)] fp8, sw [C] f32)
    with partition = ci % 128, k = ci // 128, m = cout % 128.
    """
    scale = gamma / np.sqrt(var + EPS)
    wf = (np.asarray(w, np.float32) * scale[:, None, None, None]).astype(np.float32)
    mx = np.maximum(np.abs(wf).reshape(C, -1).max(axis=1), 1e-30)
    sw = np.exp2(np.floor(np.log2(WMAX / mx))).astype(np.float32)
    ws = wf * sw[:, None, None, None]                # [o, i, ky, kx]
    a = ws.reshape(2, 128, 2, 128, 3, 3)             # [co, m, k, p, ky, kx]
    a = np.ascontiguousarray(a.transpose(3, 0, 4, 5, 2, 1))  # [p,co,ky,kx,k,m]
    a = a.reshape(128, 2, 9, 2, 128)                 # [p, co, t, k, m]
    hi = _q8(a)
    lo = _q8(a - hi.astype(np.float32))
    return np.ascontiguousarray(np.stack([hi, lo], axis=3)), sw


def pack_x(x: np.ndarray) -> np.ndarray:
    """Zero-pad 1px, scale by XS, split into e4m3 hi + residual lo.

    Output [B, 128, 2(split), 2(k), HP, WP] fp8 with partition = c % 128.
    """
    import ml_dtypes
    n, c, h, w = x.shape
    xs = (x.astype(np.float32) * XS).reshape(n, 2, 128, h, w)
    hi = _q8(xs)
    lo = _q8(xs - hi.astype(np.float32))
    out = np.zeros((n, 128, 2, 2, h + 2, w + 2), dtype=ml_dtypes.float8_e4m3)
    # [n, k, p, h, w] -> [n, p, split, k, h, w]
    out[:, :, 0, :, 1:h + 1, 1:w + 1] = hi.transpose(0, 2, 1, 3, 4)
    out[:, :, 1, :, 1:h + 1, 1:w + 1] = lo.transpose(0, 2, 1, 3, 4)
    return out


def pack_bias_scale(b_up, m_up, g_up, v_up, sw_up,
                    b_down, m_down, g_down, v_down, sw_down):
    def shift(b, m, g, v):
        return (b - m * (g / np.sqrt(v + EPS))).astype(np.float32)
    s_up = shift(b_up, m_up, g_up, v_up)
    s_dn = shift(b_down, m_down, g_down, v_down)
    bias = np.ascontiguousarray(
        np.stack([s_up[:128], s_up[128:], s_dn[:128], s_dn[128:]], axis=1))
    inv_u = (1.0 / (XS * sw_up)).astype(np.float32)
    inv_d = (1.0 / (XS * sw_down)).astype(np.float32)
    scale = np.ascontiguousarray(
        np.stack([inv_u[:128], inv_u[128:], inv_d[:128], inv_d[128:]], axis=1))
    return bias, scale


_CACHE: dict = {}


def _get_program() -> bass.Bass:
    if "nc" not in _CACHE:
        _CACHE["nc"] = build_program()
    return _CACHE["nc"]


def make_in_maps(x, w_up, g_up, b_up, m_up, v_up,
                 w_down, g_down, b_down, m_down, v_down):
    xq = pack_x(np.asarray(x, dtype=np.float32))
    wq_up, sw_up = pack_weights(np.asarray(w_up, np.float32),
                                np.asarray(g_up, np.float32),
                                np.asarray(v_up, np.float32))
    wq_dn, sw_dn = pack_weights(np.asarray(w_down, np.float32),
                                np.asarray(g_down, np.float32),
                                np.asarray(v_down, np.float32))
    # unit-major (down branch first, matching the kernel's UNITS order):
    # [p, unit, co, t, ws, k, m] -> [p, 72, 2, 128]
    wq = np.ascontiguousarray(
        np.stack([wq_dn, wq_up], axis=1)).reshape(128, 72, 2, 128)
    bias, scale = pack_bias_scale(
        np.asarray(b_up, np.float32), np.asarray(m_up, np.float32),
        np.asarray(g_up, np.float32), np.asarray(v_up, np.float32), sw_up,
        np.asarray(b_down, np.float32), np.asarray(m_down, np.float32),
        np.asarray(g_down, np.float32), np.asarray(v_down, np.float32), sw_dn)
    return [{"x": xq[i * BPC:(i + 1) * BPC], "wq": wq, "bias": bias,
             "scale": scale} for i in range(N_CORES)]


def execute(in_maps):
    nc = _get_program()
    return run_bass_kernel_spmd(nc, in_maps, list(range(N_CORES)))


def kernel(x, w_up, g_up, b_up, m_up, v_up,
           w_down, g_down, b_down, m_down, v_down) -> np.ndarray:
    in_maps = make_in_maps(x, w_up, g_up, b_up, m_up, v_up,
                           w_down, g_down, b_down, m_down, v_down)
    res = execute(in_maps)
    return np.concatenate([res.results[i]["out"] for i in range(N_CORES)], axis=0)

